# revision 20
# baseline (speedup 1.0000x reference)
"""Multi-head self-attention (RoPE, causal) on 8 Trainium2 NeuronCores.

Sharding: tensor-parallel over heads — 2 of 16 heads per core. Each core
computes its heads' Q/K/V projections (with RoPE folded in via a dual
projection against row-swapped weights), causal flash-style attention in a
transposed [dim, seq] layout, and a partial output projection against its
row-slice of w_o. The host sums the 8 partial outputs.

All heavy matmuls run as float32r (TF32-class, ~1.7e-4 rel rounding).
"""

import sys

sys.path.insert(0, "/opt/trn_rl_repo")
sys.path.insert(0, "/root/problem")

import numpy as np
import ml_dtypes

import concourse.bass as bass
import concourse.tile as tile

# ---------------------------------------------------------------------------
# Toolchain fixes (inlined, self-contained): walrus on this stack allows only
# one sync-wait per instruction; the Tile tail drain carries many.  Also the
# image's antenv lacks the NTFF profile hook.
# ---------------------------------------------------------------------------
from concourse.vector_clock import ScopedClock

MAXW = 1


def _patched_drain_and_barrier(self, tick_clock, wait_clock):
    nc = self.nc
    drain_inst = nc.sync.drain()
    wait_clock.add_sem_waits(
        drain_inst.ins, ScopedClock({None: tick_clock.global_clock})
    )
    si = drain_inst.ins.sync_info
    waits = list(si.on_wait or []) if si is not None else []
    if len(waits) > MAXW:
        si.on_wait = waits[:MAXW]
        rest = waits[MAXW:]
        while rest:
            chunk, rest = rest[:MAXW], rest[MAXW:]
            nop = nc.sync.nop(nofuse=True)
            nsi = nop.ins.sync_info
            if nsi is None:
                import bass_rust

                nop.ins.sync_info = bass_rust.SyncInfo(on_wait=chunk, on_update=[])
            else:
                nsi.on_wait = list(nsi.on_wait or []) + chunk

    nc.all_engine_barrier()
    assert self.sems is not None
    popped = nc._tile_sem_poison_stack.pop()
    assert popped is self._sem_poison
    nc.clear_and_free_semaphores(list(self.sems.allocated().values()))
    nc.all_engine_barrier()


def apply():
    tile.TileContext._drain_and_barrier = _patched_drain_and_barrier
    _install_ntff_hook_shim()
    _install_compile_hook()


def _split_waits_json(bir_json: bytes) -> bytes:
    """Walrus on this toolchain allows at most one sync-wait per instruction.
    Insert a same-engine NoOp carrying each excess wait immediately before any
    multi-wait instruction (engine blocks at the NoOp instead — identical
    semantics, order preserved)."""
    import json as _json

    d = _json.loads(bir_json)
    n_split = 0
    for fn in d.get("functions", []):
        for bb in fn.get("blocks", []):
            insts = bb.get("instructions", [])
            out = []
            for inst in insts:
                si = inst.get("sync_info")
                waits = (si or {}).get("on_wait") or []
                if len(waits) > 1:
                    ge = [w for w in waits if w.get("wait_mode") == "sem-ge-imm"]
                    other = [w for w in waits if w.get("wait_mode") != "sem-ge-imm"]
                    # keep one wait on the instruction (prefer a non-ge if present)
                    if other:
                        keep = other
                        move = ge
                    else:
                        keep = ge[-1:]
                        move = ge[:-1]
                    if len(keep) <= 1 and move:
                        for i, w in enumerate(move):
                            out.append(
                                {
                                    "debug": inst.get("debug", 0),
                                    "engine": inst["engine"],
                                    "ins": [],
                                    "outs": [],
                                    "name": f"{inst['name']}-ws{i}",
                                    "opcode": "NoOp",
                                    "sync_info": {"on_update": [], "on_wait": [w]},
                                }
                            )
                            n_split += 1
                        si["on_wait"] = keep
                out.append(inst)
            bb["instructions"] = out
    if n_split:
        print(f"tilefix: split {n_split} excess waits onto NoOps")
    return _json.dumps(d).encode()


def _install_compile_hook():
    import concourse.bass_utils as bu
    import concourse.bass2jax as b2j

    if getattr(bu, "_tilefix_wrapped", False):
        return
    orig = bu.compile_bir_kernel

    def wrapped(bir_json, tmpdir, neff_name="file.neff"):
        return orig(_split_waits_json(bir_json), tmpdir, neff_name)

    bu.compile_bir_kernel = wrapped
    b2j.compile_bir_kernel = wrapped
    bu._tilefix_wrapped = True


def _install_ntff_hook_shim():
    """The image's antenv package lacks axon_hooks; provide a stand-in module
    exposing the ctypes-based NTFF profile hook against /opt/axon/libaxon_pjrt.so
    so run_bass_kernel_spmd(trace=True) works."""
    import sys as _sys
    import types

    if "antenv.axon_hooks" in _sys.modules:
        return
    mod = types.ModuleType("antenv.axon_hooks")
    _state = {"hook": None}

    so_path = "/opt/axon/libaxon_pjrt.so"
    try:
        import importlib.util

        spec = importlib.util.spec_from_file_location(
            "trn_agent_boot.trn_boot", "/root/.axon_site/trn_agent_boot/trn_boot.py"
        )
        # trn_boot is already importable as a package in the axon site; reuse it.
        import trn_agent_boot.trn_boot as _tb  # type: ignore

        _state["hook"] = _tb._ntff_profile_via_ctypes(so_path)
    except Exception:
        _state["hook"] = None

    def get_axon_ntff_profile_hook():
        return _state["hook"]

    def set_axon_ntff_profile_hook(h):
        _state["hook"] = h

    mod.get_axon_ntff_profile_hook = get_axon_ntff_profile_hook
    mod.set_axon_ntff_profile_hook = set_axon_ntff_profile_hook
    _sys.modules["antenv.axon_hooks"] = mod

apply()

from concourse import mybir
from concourse.bass_utils import run_bass_kernel_spmd
from concourse.masks import make_identity

F32 = mybir.dt.float32
F32R = mybir.dt.float32r
BF16 = mybir.dt.bfloat16
EXP = mybir.ActivationFunctionType.Exp
LN = mybir.ActivationFunctionType.Ln

S = 4096          # sequence length
D = 1024          # model dim
NH = 16           # heads
HD = 64           # head dim
NCORES = 8
HPC = NH // NCORES  # heads per core = 2
QT = 512          # qpos tile (free dim of S^T / PV matmuls)
KC = 128          # kpos chunk (partition dim of S^T tiles)
NQT = S // QT     # 8
NKC = S // KC     # 32
NEG = -1.0e30

_CACHE = {}


def _build_nc():
    nc = bass.Bass("TRN2")

    xT_d = nc.dram_tensor("xT", [D, S], BF16, kind="ExternalInput")
    wqa_d = nc.dram_tensor("wqa", [D, 128], BF16, kind="ExternalInput")
    wqb_d = nc.dram_tensor("wqb", [D, 128], BF16, kind="ExternalInput")
    wka_d = nc.dram_tensor("wka", [D, 128], BF16, kind="ExternalInput")
    wkb_d = nc.dram_tensor("wkb", [D, 128], BF16, kind="ExternalInput")
    wv_d = nc.dram_tensor("wv", [D, 128], BF16, kind="ExternalInput")
    wo_d = nc.dram_tensor("wo", [128, D], F32R, kind="ExternalInput")
    cos_d = nc.dram_tensor("cosP", [128, S], F32, kind="ExternalInput")
    sin_d = nc.dram_tensor("sinPs", [128, S], F32, kind="ExternalInput")
    out_d = nc.dram_tensor("out", [S, D], F32, kind="ExternalOutput")

    with tile.TileContext(nc) as tc:
        with (
            tc.tile_pool(name="const", bufs=1) as cpool,
            tc.tile_pool(name="big", bufs=1) as big,
            tc.tile_pool(name="xp", bufs=3) as xp,
            tc.tile_pool(name="pt", bufs=5) as ptp,
            tc.tile_pool(name="ob", bufs=4) as obp,
            tc.tile_pool(name="sm", bufs=3) as smp,
            tc.tile_pool(name="ps", bufs=2, space="PSUM") as ps,
            tc.tile_pool(name="pss", bufs=2, space="PSUM") as pssp,
            tc.tile_pool(name="po", bufs=2, space="PSUM") as po,
        ):
            # ---- constants -------------------------------------------------
            ident = cpool.tile([128, 128], F32)
            make_identity(nc, ident[:])
            identb = cpool.tile([128, 128], BF16)
            make_identity(nc, identb[:])
            cosP = cpool.tile([128, S], F32)
            nc.sync.dma_start(cosP[:], cos_d.ap())
            sinPs = cpool.tile([128, S], F32)
            nc.sync.dma_start(sinPs[:], sin_d.ap())
            ones1 = cpool.tile([1, 64], F32R)
            nc.scalar.activation(
                ones1[:], cosP[0:1, 0:64],
                mybir.ActivationFunctionType.Copy, bias=1.0, scale=0.0,
            )

            w_sb = {}
            for name, dram in (
                ("qa", wqa_d), ("qb", wqb_d), ("ka", wka_d), ("kb", wkb_d),
                ("v", wv_d),
            ):
                t = cpool.tile([128, 8, 128], BF16, name=f"w_{name}")
                nc.sync.dma_start(
                    t[:], dram.ap().rearrange("(kc p) m -> p kc m", p=128)
                )
                w_sb[name] = t
            wo_sb = cpool.tile([128, D], F32R)
            nc.sync.dma_start(wo_sb[:], wo_d.ap())

            # ---- warmups: make each engine observe const DMAs (1-wait rule)
            junk = ps.tile([128, QT], F32, tag="ps")
            for t in w_sb.values():
                nc.tensor.matmul(junk[:, 0:128], t[:, 0, :], t[:, 0, :],
                                 start=True, stop=True)
            nc.tensor.matmul(junk[:, 0:128], wo_sb[:, 0:128], wo_sb[:, 0:128],
                             start=True, stop=True)
            nc.tensor.transpose(junk[:, 0:128], ident[:], ident[:])
            nc.tensor.transpose(junk[:].bitcast(BF16)[:, 0:128], identb[:], identb[:])
            tch = smp.tile([128, 4], F32, tag="touch")
            nc.vector.tensor_copy(tch[:, 0:1], cosP[:, 0:1])
            nc.vector.tensor_copy(tch[:, 1:2], sinPs[:, 0:1])

            # ---- persistent activations -----------------------------------
            qT = big.tile([128, S], BF16)   # rope'd q, [2*64 dims, seq]
            kT = big.tile([128, S], BF16)
            attnT = big.tile([128, S], F32R)  # normalized attn out, [dims, seq]
            vaug = [big.tile([128, NKC * 65], BF16, name=f"vaug{h}") for h in range(HPC)]
            for h in range(HPC):
                va = vaug[h].rearrange("p (kc m) -> p kc m", m=65)
                nc.scalar.activation(
                    va[:, :, 64:65], cosP[:, 0:NKC],
                    mybir.ActivationFunctionType.Copy, bias=1.0, scale=0.0,
                )

            # ---- phase 1 body: projections + rope + v transpose -----------
            def proj_step(st):
                sl = slice(st * QT, (st + 1) * QT)
                xblk = xp.tile([128, 8, QT], BF16, tag="xblk")
                for kc in range(8):
                    nc.sync.dma_start(
                        xblk[:, kc, :],
                        xT_d.ap()[kc * 128:(kc + 1) * 128, sl],
                    )

                def proj(wname):
                    psum = ps.tile([128, QT], F32, tag="ps")
                    for kc in range(8):
                        nc.tensor.matmul(
                            psum[:], w_sb[wname][:, kc, :], xblk[:, kc, :],
                            start=(kc == 0), stop=(kc == 7),
                        )
                    return psum

                for dst in (qT, kT):
                    wa, wb = ("qa", "qb") if dst is qT else ("ka", "kb")
                    pa = proj(wa)
                    nc.vector.tensor_mul(dst[:, sl], pa[:], cosP[:, sl])
                    pb = proj(wb)
                    tmp = smp.tile([128, QT], BF16, tag="ropetmp")
                    nc.vector.tensor_mul(tmp[:], pb[:], sinPs[:, sl])
                    nc.vector.tensor_add(dst[:, sl], dst[:, sl], tmp[:])

                pv_ = proj("v")
                vtmp = smp.tile([128, QT], BF16, tag="vtmp")
                nc.vector.tensor_copy(vtmp[:], pv_[:])
                for z in range(4):
                    kc = st * 4 + z
                    pst = po.tile([128, 128], BF16, tag="po")
                    nc.tensor.transpose(
                        pst[:], vtmp[:, z * 128:(z + 1) * 128], identb[:]
                    )
                    for h in range(HPC):
                        va = vaug[h].rearrange("p (kc m) -> p kc m", m=65)
                        nc.vector.tensor_copy(
                            va[:, kc, 0:64], pst[:, h * 64:(h + 1) * 64]
                        )

            # ---- phase 2 body: attention + output projection --------------
            def attn_step(qt):
                qsl = slice(qt * QT, (qt + 1) * QT)
                po_h = [po.tile([128, QT], F32, tag="po", name=f"po{qt}_{h}") for h in range(HPC)]
                last_kc = 4 * qt + 3
                for kc in range(last_kc + 1):
                    ksl = slice(kc * 128, (kc + 1) * 128)
                    joff = max(0, (kc - 4 * qt)) * 128  # skipped cols on diag
                    ps_s = pssp.tile([128, 2 * QT], F32, tag="pss")
                    for h in range(HPC):
                        hsl = slice(h * 64, (h + 1) * 64)
                        nc.tensor.matmul(
                            ps_s[:, h * QT + joff:(h + 1) * QT],
                            kT[hsl, ksl],
                            qT[hsl, qt * QT + joff:(qt + 1) * QT],
                            start=True, stop=True,
                        )
                    pt_t = ptp.tile([128, 2 * QT], BF16, tag="pt")
                    ps3 = ps_s[:].rearrange("p (h q) -> p h q", h=HPC)
                    pt3 = pt_t[:].rearrange("p (h q) -> p h q", h=HPC)
                    nc.scalar.activation(
                        pt3[:, :, joff:QT], ps3[:, :, joff:QT], EXP, scale=0.125
                    )
                    if kc >= 4 * qt:
                        j = kc - 4 * qt
                        for h in range(HPC):
                            nc.gpsimd.affine_select(
                                out=pt_t[:, h * QT:(h + 1) * QT],
                                in_=pt_t[:, h * QT:(h + 1) * QT],
                                compare_op=mybir.AluOpType.is_ge,
                                fill=0.0, base=-(j * 128),
                                pattern=[[1, QT]], channel_multiplier=-1,
                            )
                    for h in range(HPC):
                        va = vaug[h].rearrange("p (kc m) -> p kc m", m=65)
                        nc.tensor.matmul(
                            po_h[h][0:65, joff:QT], va[:, kc, :],
                            pt_t[:, h * QT + joff:(h + 1) * QT],
                            start=(kc == 0), stop=(kc == last_kc),
                        )

                tlns, rcps, pbs, rbs = [], [], [], []
                for h in range(HPC):
                    tln = smp.tile([1, QT], F32, tag="tln", name=f"tln{qt}_{h}")
                    nc.scalar.activation(tln[:], po_h[h][64:65, :], LN)
                    tlns.append(tln)
                for h in range(HPC):
                    rcp = smp.tile([1, QT], F32R, tag="rcp", name=f"rcp{qt}_{h}")
                    nc.scalar.activation(rcp[:], tlns[h][:], EXP, scale=-1.0)
                    rcps.append(rcp)
                for h in range(HPC):
                    pb_ps = ps.tile([128, QT], F32, tag="ps", name=f"pb{qt}_{h}")
                    nc.tensor.matmul(
                        pb_ps[0:64, :], ones1[:], rcps[h][:],
                        start=True, stop=True,
                    )
                    pbs.append(pb_ps)
                for h in range(HPC):
                    rb = smp.tile([64, QT], F32, tag="rb", name=f"rb{qt}_{h}")
                    nc.vector.tensor_copy(rb[:], pbs[h][0:64, :])
                    rbs.append(rb)
                for h in range(HPC):
                    hsl = slice(h * 64, (h + 1) * 64)
                    nc.vector.tensor_mul(
                        attnT[hsl, qsl], po_h[h][0:64, :], rbs[h][:]
                    )

                for z in range(4):
                    csl = slice(qt * QT + z * 128, qt * QT + (z + 1) * 128)
                    for ncol in range(2):
                        osl = slice(ncol * 512, (ncol + 1) * 512)
                        ps_o = ps.tile([128, QT], F32, tag="ps")
                        nc.tensor.matmul(
                            ps_o[:], attnT[:, csl], wo_sb[:, osl],
                            start=True, stop=True,
                        )
                        osb = obp.tile([128, QT], F32, tag="ob")
                        nc.vector.tensor_copy(osb[:], ps_o[:])
                        nc.sync.dma_start(out_d.ap()[csl, osl], osb[:])

            for step in range(NQT):
                proj_step(step)
            for step in range(NQT - 1, -1, -1):
                attn_step(step)

    return nc


def _rope_tables(token_positions):
    """cosP/sinPs in the transposed per-partition layout.

    Row r (r in 0..127): head = r//64, idx = r%64; pair j = idx%32.
    Rows with idx<32 hold even rope dims (d=2j), idx>=32 odd dims (d=2j+1).
    sinPs is the *swap-adjusted* sin table: the swapped projection qB holds
    the partner value (q_odd on even rows, q_even on odd rows), so even rows
    need -sin (r_e = q_e cos - q_o sin) and odd rows +sin (r_o = q_o cos +
    q_e sin).
    """
    pos = token_positions.astype(np.float32)  # [S]
    inv = (1.0 / (10000.0 ** (np.arange(0, HD, 2, dtype=np.float32) / HD)))
    freqs = pos[:, None] * inv[None, :]        # [S, 32]
    cos32 = np.cos(freqs).T.astype(np.float32)  # [32, S]
    sin32 = np.sin(freqs).T.astype(np.float32)
    cosP = np.concatenate([cos32, cos32, cos32, cos32], 0)
    sinPs = np.concatenate([-sin32, sin32, -sin32, sin32], 0)
    return np.ascontiguousarray(cosP), np.ascontiguousarray(sinPs)


def _bias_tiles():
    b = np.zeros((4, 128, QT), np.float32)
    p = np.arange(128)[:, None]
    f = np.arange(QT)[None, :]
    for j in range(4):
        b[j] = np.where(f >= p + j * 128, 0.0, NEG)
    return b


def kernel(x, w_q, w_k, w_v, w_o, token_positions):
    x = np.asarray(x, dtype=np.float32)
    w_q = np.asarray(w_q, dtype=np.float32)
    w_k = np.asarray(w_k, dtype=np.float32)
    w_v = np.asarray(w_v, dtype=np.float32)
    w_o = np.asarray(w_o, dtype=np.float32)
    tp = np.asarray(token_positions).reshape(-1)

    b = x.shape[0]
    assert x.shape == (b, S, D) and b == 1

    xT = np.ascontiguousarray(x[0].T).astype(ml_dtypes.bfloat16)  # [D, S]
    cosP, sinPs = _rope_tables(tp)

    # per-head permutation: evens (0,2,..62) then odds (1,3,..63)
    perm64 = np.concatenate([np.arange(0, HD, 2), np.arange(1, HD, 2)])
    # swap of the 32-blocks within each head: [32:64, 0:32] per 64-block
    swap128 = np.concatenate([
        np.arange(32, 64), np.arange(0, 32),
        np.arange(96, 128), np.arange(64, 96),
    ])

    if "nc" not in _CACHE:
        _CACHE["nc"] = _build_nc()
    nc = _CACHE["nc"]

    in_maps = []
    for c in range(NCORES):
        rows = np.concatenate(
            [c * 128 + h * 64 + perm64 for h in range(HPC)]
        )  # 128 permuted q/k output dims of this core
        wq_p = w_q[rows]                    # [128, D]
        wk_p = w_k[rows]
        in_maps.append({
            "xT": xT,
            "wqa": np.ascontiguousarray(wq_p.T).astype(ml_dtypes.bfloat16),
            "wqb": np.ascontiguousarray(wq_p[swap128].T).astype(ml_dtypes.bfloat16),
            "wka": np.ascontiguousarray(wk_p.T).astype(ml_dtypes.bfloat16),
            "wkb": np.ascontiguousarray(wk_p[swap128].T).astype(ml_dtypes.bfloat16),
            "wv": np.ascontiguousarray(w_v[c * 128:(c + 1) * 128].T).astype(ml_dtypes.bfloat16),
            "wo": np.ascontiguousarray(w_o[:, c * 128:(c + 1) * 128].T),
            "cosP": cosP,
            "sinPs": sinPs,
        })

    _CACHE["last_in_maps"] = in_maps
    res = run_bass_kernel_spmd(nc, in_maps, core_ids=list(range(NCORES)))
    out = res.results[0]["out"].astype(np.float64)
    for c in range(1, NCORES):
        out += res.results[c]["out"]
    return out.astype(np.float32)[None]


if __name__ == "__main__":
    rng = np.random.default_rng(0)
    x = rng.standard_normal((1, S, D), dtype=np.float32)
    sc = 1.0 / np.sqrt(D)
    wq = rng.standard_normal((D, D), dtype=np.float32) * sc
    wk = rng.standard_normal((D, D), dtype=np.float32) * sc
    wv = rng.standard_normal((D, D), dtype=np.float32) * sc
    wo = rng.standard_normal((D, D), dtype=np.float32) * sc
    tpos = np.arange(S, dtype=np.int32)[None]
    out = kernel(x=x, w_q=wq, w_k=wk, w_v=wv, w_o=wo, token_positions=tpos)
    print("kernel out:", out.shape, out.dtype, float(np.abs(out).max()))
